# revision 18
# baseline (speedup 1.0000x reference)
"""Trainium2 Bass kernel for nn_CDA_attention (density-modulated attention).

Contract: kernel(**full_inputs) -> full output [8, 256, 64, 64] float32.
Data-parallel over batch: core b computes batch b.

Per-core computation (batch b, C=256, N=4096):
  - density chain (Laplacian -> conv(1->8) -> relu -> conv(8->1) -> sigmoid)
    in [64,64] image layout on DVE/ACT, producing per-key scale
    skv[nk] = C^-0.5 / (1 + 2*(1-density)).
  - q,k = Wq/Wk @ x  (float32r matmuls, [c,n] layout)
  - vproj = (out_w @ Wv) @ x, transposed layout [nk, c] with a ones column
    appended so attn@vproj also yields softmax row-sums.
  - scores computed transposed: sT[nk, nq] = k^T q; exp fused with the
    per-key scale as a per-partition activation scale; softmax division by
    the row-sum after the attn@vproj matmul.
  - final transpose back to [c, n] on the PE, + fused bias + residual.
"""

import os
import sys

sys.path.insert(0, "/opt/trn_rl_repo")

from contextlib import ExitStack

import numpy as np

import concourse.bass as bass
import concourse.mybir as mybir
import concourse.tile as tile
from concourse import bacc, bass_utils
from concourse.masks import make_identity

B, C, HH, WW = 8, 256, 64, 64
N = HH * WW          # 4096
P = 128
CC = C // P          # 2 channel chunks
NQT = 512            # query tile (columns per QK^T matmul)
NQ_TILES = N // NQT  # 8
NKC = N // P         # 32 key chunks
NSUB = NQT // P      # 4 query sub-tiles per query tile

f32 = mybir.dt.float32
f32r = mybir.dt.float32r
bf16 = mybir.dt.bfloat16
AF = mybir.ActivationFunctionType
ALU = mybir.AluOpType

# tap order for 3x3 convs: center first so the first tap writes the full tile
TAPS = [(1, 1)] + [(ky, kx) for ky in range(3) for kx in range(3) if (ky, kx) != (1, 1)]


def _make_row_shifted(nc, pool, src, name):
    """Return {dy: AP} of row-shifted copies of src ([64, ...] SBUF tile):
    sh[+1][p] = src[p+1] (last row 0), sh[-1][p] = src[p-1] (first row 0).
    Compute engines need 32-aligned partition bases, DMA does not — so the
    row shift is done once by DMA into zeroed tiles and every conv tap then
    reads/writes full partition ranges."""
    shape = list(src.shape)
    p1 = pool.tile(shape, f32, name=f"{name}_p1")
    m1 = pool.tile(shape, f32, name=f"{name}_m1")
    nc.vector.memset(p1[:], 0.0)
    nc.vector.memset(m1[:], 0.0)
    nc.sync.dma_start(p1[0:63], src[1:64])
    nc.sync.dma_start(m1[1:64], src[0:63])
    return {0: src, 1: p1, -1: m1}


def _conv3x3(nc, out_t, var_slices, wcol_fn, n_in):
    """out[h,w] = sum over (ic, ky, kx) of w[ic,ky,kx] * in[ic, h+ky-1, w+kx-1].

    var_slices(ic, dy, cs) -> [64, len] AP of the dy-row-shifted variant of
    input channel ic, columns cs.  wcol_fn(ic, t) -> [64,1] weight AP.
    First tap (center) initializes out_t.
    """
    first = True
    for ic in range(n_in):
        for ky, kx in TAPS:
            dy, dx = ky - 1, kx - 1
            c0, c1 = max(0, -dx), WW - max(0, dx)
            src = var_slices(ic, dy, slice(c0 + dx, c1 + dx))
            wcol = wcol_fn(ic, ky * 3 + kx)
            if first:
                assert (dy, dx) == (0, 0)
                nc.vector.tensor_scalar(
                    out=out_t[:, :], in0=src, scalar1=wcol, scalar2=None,
                    op0=ALU.mult)
                first = False
            else:
                dst = out_t[:, c0:c1]
                nc.vector.scalar_tensor_tensor(
                    out=dst, in0=src, scalar=wcol, in1=dst,
                    op0=ALU.mult, op1=ALU.add)


def build_kernel_body(tc, ctx, d):
    nc = tc.nc
    x_d, wqk_d, wvo_d, qkb_d, bfin_d = d["x"], d["wqk"], d["wvo"], d["qkb"], d["bfin"]
    w1b_d, w1bias_d, w2b_d, w2bias_d = d["w1b"], d["w1bias"], d["w2b"], d["w2bias"]
    out_d, scr1, scr2 = d["out"], d["scr1"], d["scr2"]

    const = ctx.enter_context(tc.tile_pool(name="const", bufs=1))
    big = ctx.enter_context(tc.tile_pool(name="big", bufs=1))
    ps_pool = ctx.enter_context(tc.tile_pool(name="ps", bufs=2, space="PSUM"))
    po_pool = ctx.enter_context(tc.tile_pool(name="po", bufs=4, space="PSUM"))
    pt_pool = ctx.enter_context(tc.tile_pool(name="pt", bufs=2, space="PSUM"))
    fin_pool = ctx.enter_context(tc.tile_pool(name="fin", bufs=2))
    osb_pool = ctx.enter_context(tc.tile_pool(name="osb", bufs=3))
    rcp_pool = ctx.enter_context(tc.tile_pool(name="rcp", bufs=4))

    # ---- persistent SBUF tiles ----
    x_sb = big.tile([P, CC, N], f32r)
    q_sb = big.tile([P, CC, N], f32r)
    k_sb = big.tile([P, CC, N], f32r)
    vproj_sb = big.tile([P, NKC, C + 1], bf16)
    exp_sb = big.tile([P, NKC, NQT], bf16)
    wqk_sb = const.tile([P, CC, 2 * C], f32r)
    wvo_sb = const.tile([P, CC, C], f32r)
    qkb_sb = const.tile([P, 4], f32)
    bfin_sb = const.tile([P, 2], f32)
    w1b_sb = const.tile([64, 72], f32)
    w1bias_sb = const.tile([64, 8], f32)
    w2b_sb = const.tile([64, 72], f32)
    w2bias_sb = const.tile([64, 1], f32)
    ident = const.tile([P, P], f32)
    ones_sb = const.tile([P, 1], f32r)
    sk_sb = const.tile([P, NKC], f32)
    gray_row = const.tile([1, N], f32)
    gray_img = const.tile([64, 64], f32)
    lap_t = const.tile([64, 64], f32)
    abs_t = const.tile([64, 64], f32)
    h1_t = const.tile([64, 8, 64], f32)
    h1r_t = const.tile([64, 8, 64], f32)
    dl_t = const.tile([64, 64], f32)
    sig_t = const.tile([64, 64], f32)
    skv_t = const.tile([64, 64], f32)

    # ---- input DMAs ----
    for ci in range(CC):
        nc.sync.dma_start(x_sb[:, ci, :], x_d[ci * P:(ci + 1) * P, :])
        nc.sync.dma_start(wqk_sb[:, ci, :], wqk_d[ci * P:(ci + 1) * P, :])
        nc.sync.dma_start(wvo_sb[:, ci, :], wvo_d[ci * P:(ci + 1) * P, :])
    nc.sync.dma_start(ones_sb[:, :], d["ones"][:, :])
    nc.sync.dma_start(qkb_sb[:, :], qkb_d[:, :])
    nc.sync.dma_start(bfin_sb[:, :], bfin_d[:, :])
    nc.sync.dma_start(w1b_sb[:, :], w1b_d[:, :])
    nc.sync.dma_start(w1bias_sb[:, :], w1bias_d[:, :])
    nc.sync.dma_start(w2b_sb[:, :], w2b_d[:, :])
    nc.sync.dma_start(w2bias_sb[:, :], w2bias_d[:, :])

    make_identity(nc, ident)
    nc.vector.memset(vproj_sb[:, :, C:C + 1], 1.0)    # ones column -> row sums

    # ---- gray = mean_c x  (PE, M=1) ----
    for nt in range(NQ_TILES):
        pg = ps_pool.tile([1, NQT], f32, tag="ps")
        for ci in range(CC):
            nc.tensor.matmul(
                pg[:, :],
                ones_sb[:, :],
                x_sb[:, ci, nt * NQT:(nt + 1) * NQT],
                start=(ci == 0), stop=(ci == CC - 1))
        nc.vector.tensor_copy(gray_row[:, nt * NQT:(nt + 1) * NQT], pg[:, :])

    # reshape [1, 4096] -> [64, 64] via DRAM
    nc.sync.dma_start(scr1.rearrange("(a b) -> a b", a=1), gray_row[:, :])
    nc.sync.dma_start(gray_img[:, :], scr1.rearrange("(h w) -> h w", w=64))

    # ---- density chain (image layout) ----
    # Laplacian: 4*g - up - down - left - right (zero SAME padding)
    gvar = _make_row_shifted(nc, const, gray_img, "gray")
    nc.vector.tensor_scalar(
        out=lap_t[:, :], in0=gray_img[:, :], scalar1=4.0, scalar2=None, op0=ALU.mult)
    for dy in (1, -1):  # out[h] += -g[h+dy]
        nc.vector.scalar_tensor_tensor(
            out=lap_t[:, :], in0=gvar[dy][:, :], scalar=-1.0, in1=lap_t[:, :],
            op0=ALU.mult, op1=ALU.add)
    for dx in (1, -1):
        c0, c1 = max(0, -dx), WW - max(0, dx)
        dst = lap_t[:, c0:c1]
        nc.vector.scalar_tensor_tensor(
            out=dst, in0=gray_img[:, c0 + dx:c1 + dx], scalar=-1.0, in1=dst,
            op0=ALU.mult, op1=ALU.add)
    nc.scalar.activation(abs_t[:, :], lap_t[:, :], AF.Abs)

    # conv1: 1 -> 8 channels; relu with bias
    avar = _make_row_shifted(nc, const, abs_t, "abs")
    for oc in range(8):
        _conv3x3(
            nc, h1_t[:, oc, :],
            lambda ic, dy, cs: avar[dy][:, cs],
            lambda ic, t, oc=oc: w1b_sb[:, oc * 9 + t:oc * 9 + t + 1],
            n_in=1)
    for oc in range(8):
        nc.scalar.activation(
            h1r_t[:, oc, :], h1_t[:, oc, :], AF.Relu,
            bias=w1bias_sb[:, oc:oc + 1])

    # conv2: 8 -> 1 channel; sigmoid with bias
    hvar = _make_row_shifted(nc, const, h1r_t, "h1r")
    _conv3x3(
        nc, dl_t,
        lambda ic, dy, cs: hvar[dy][:, ic, cs],
        lambda ic, t: w2b_sb[:, ic * 9 + t:ic * 9 + t + 1],
        n_in=8)
    nc.scalar.activation(sig_t[:, :], dl_t[:, :], AF.Sigmoid, bias=w2bias_sb[:, 0:1])
    # skv = C^-0.5 / (3 - 2*sigmoid)
    nc.scalar.activation(dl_t[:, :], sig_t[:, :], AF.Copy, bias=3.0, scale=-2.0)
    nc.vector.reciprocal(sig_t[:, :], dl_t[:, :])
    nc.vector.tensor_scalar(
        out=skv_t[:, :], in0=sig_t[:, :], scalar1=float(C) ** -0.5, scalar2=None,
        op0=ALU.mult)
    # reshape [64,64] -> [128, 32] (partition p of col j = key n = j*128+p)
    nc.sync.dma_start(scr2.rearrange("(h w) -> h w", w=64), skv_t[:, :])
    nc.sync.dma_start(sk_sb[:, :], scr2.rearrange("(j p) -> p j", p=P))

    # ---- q, k projections (float32r) ----
    for m in range(4):                   # c_out chunks: q0, q1, k0, k1
        dst = q_sb if m < 2 else k_sb
        mm = m % 2
        for nt in range(NQ_TILES):
            pq = ps_pool.tile([P, NQT], f32, tag="ps")
            for ci in range(CC):
                nc.tensor.matmul(
                    pq[:, :],
                    wqk_sb[:, ci, m * P:(m + 1) * P],
                    x_sb[:, ci, nt * NQT:(nt + 1) * NQT],
                    start=(ci == 0), stop=(ci == CC - 1))
            nc.vector.tensor_scalar(          # psum evict + bias, rounds to f32r
                out=dst[:, mm, nt * NQT:(nt + 1) * NQT], in0=pq[:, :],
                scalar1=qkb_sb[:, m:m + 1], scalar2=None, op0=ALU.add)

    # ---- vproj = (Wout @ Wv) x, transposed [nk, c] (float32r -> bf16) ----
    for j in range(NKC):
        pv = po_pool.tile([P, C], f32, tag="po")
        for ci in range(CC):
            nc.tensor.matmul(
                pv[:, :],
                x_sb[:, ci, j * P:(j + 1) * P],
                wvo_sb[:, ci, :],
                start=(ci == 0), stop=(ci == CC - 1))
        nc.vector.tensor_copy(vproj_sb[:, j, 0:C], pv[:, :])

    # ---- attention ----
    for it in range(NQ_TILES):
        nq0 = it * NQT
        # scores (transposed) + fused exp(scale * s)
        for j in range(NKC):
            ps = ps_pool.tile([P, NQT], f32, tag="ps")
            for ci in range(CC):
                nc.tensor.matmul(
                    ps[:, :],
                    k_sb[:, ci, j * P:(j + 1) * P],
                    q_sb[:, ci, nq0:nq0 + NQT],
                    start=(ci == 0), stop=(ci == CC - 1))
            nc.scalar.activation(
                exp_sb[:, j, :], ps[:, :], AF.Exp, scale=sk_sb[:, j:j + 1])

        # attn @ [vproj | 1]
        pos = []
        for s in range(NSUB):
            po = po_pool.tile([P, C + 1], f32, tag="po")
            pos.append(po)
        for j in range(NKC):
            for s in range(NSUB):
                nc.tensor.matmul(
                    pos[s][:, :],
                    exp_sb[:, j, s * P:(s + 1) * P],
                    vproj_sb[:, j, :],
                    start=(j == 0), stop=(j == NKC - 1))

        fin = fin_pool.tile([P, CC, NQT], f32)
        for s in range(NSUB):
            rcp = rcp_pool.tile([P, 1], f32)
            nc.vector.reciprocal(rcp[:, :], pos[s][:, C:C + 1])
            osb = osb_pool.tile([P, C], f32)
            nc.vector.tensor_scalar(   # softmax normalization (divide by rowsum)
                out=osb[:, :], in0=pos[s][:, 0:C], scalar1=rcp[:, :], scalar2=None,
                op0=ALU.mult)
            for ci in range(CC):
                pt = pt_pool.tile([P, P], f32, tag="pt")
                nc.tensor.transpose(pt[:, :], osb[:, ci * P:(ci + 1) * P], ident[:, :])
                nc.vector.scalar_tensor_tensor(
                    out=fin[:, ci, s * P:(s + 1) * P],
                    in0=pt[:, :],
                    scalar=bfin_sb[:, ci:ci + 1],
                    in1=x_sb[:, ci, nq0 + s * P:nq0 + (s + 1) * P].bitcast(f32),
                    op0=ALU.add, op1=ALU.add)
        for ci in range(CC):
            nc.sync.dma_start(out_d[ci * P:(ci + 1) * P, nq0:nq0 + NQT], fin[:, ci, :])


def build_nc():
    nc = bacc.Bacc("TRN2", target_bir_lowering=False, debug=False)
    d = {}
    def inp(name, shape, dt=f32):
        d[name] = nc.dram_tensor(name, shape, dt, kind="ExternalInput").ap()
    inp("x", (C, N), f32r)      # f32r so a plain DMA is a legal f32r producer
    inp("wqk", (C, 2 * C), f32r)
    inp("wvo", (C, C), f32r)
    inp("ones", (P, 1), f32r)   # 1/C column (folds the channel mean)
    inp("qkb", (P, 4))
    inp("bfin", (P, 2))
    inp("w1b", (64, 72))
    inp("w1bias", (64, 8))
    inp("w2b", (64, 72))
    inp("w2bias", (64, 1))
    d["out"] = nc.dram_tensor("out", (C, N), f32, kind="ExternalOutput").ap()
    d["scr1"] = nc.dram_tensor("scr1", (N,), f32, kind="Internal").ap()
    d["scr2"] = nc.dram_tensor("scr2", (N,), f32, kind="Internal").ap()

    with tile.TileContext(nc) as tc, ExitStack() as ctx:
        build_kernel_body(tc, ctx, d)
    nc.compile()
    return nc


def host_inputs(x, qkv_w, qkv_b, out_w, out_b, d1_w, d1_b, d2_w, d2_b):
    f = np.float32
    x = np.asarray(x, f)
    wq = np.asarray(qkv_w, f)[:, :, 0, 0]          # [768, 256]
    qkv_b = np.asarray(qkv_b, f)
    wout = np.asarray(out_w, f)[:, :, 0, 0]        # [256, 256]
    out_b = np.asarray(out_b, f)
    shared = {
        "wqk": np.ascontiguousarray(wq[0:2 * C].T),
        "wvo": np.ascontiguousarray((wout @ wq[2 * C:3 * C]).T),
        "qkb": np.ascontiguousarray(qkv_b[0:2 * C].reshape(4, P).T),
        "bfin": np.ascontiguousarray(
            (wout @ qkv_b[2 * C:3 * C] + out_b).reshape(2, P).T),
        "w1b": np.tile(np.asarray(d1_w, f).reshape(1, 72), (64, 1)),
        "w1bias": np.tile(np.asarray(d1_b, f).reshape(1, 8), (64, 1)),
        "w2b": np.tile(np.asarray(d2_w, f).reshape(1, 72), (64, 1)),
        "w2bias": np.tile(np.asarray(d2_b, f).reshape(1, 1), (64, 1)),
        "ones": np.full((P, 1), 1.0 / C, f),
    }
    shared = {k: np.ascontiguousarray(v, dtype=f) for k, v in shared.items()}
    xs = x.reshape(B, C, N)
    return [dict(x=np.ascontiguousarray(xs[b]), **shared) for b in range(B)]


_NC_CACHE = {}


def _get_nc():
    if "nc" not in _NC_CACHE:
        _NC_CACHE["nc"] = build_nc()
    return _NC_CACHE["nc"]


def kernel(x, qkv_w, qkv_b, out_w, out_b, d1_w, d1_b, d2_w, d2_b):
    in_maps = host_inputs(x, qkv_w, qkv_b, out_w, out_b, d1_w, d1_b, d2_w, d2_b)
    nc = _get_nc()
    trace = bool(int(os.environ.get("KERNEL_TRACE", "0")))
    res = bass_utils.run_bass_kernel_spmd(
        nc, in_maps, core_ids=list(range(B)), trace=trace)
    _NC_CACHE["last_results"] = res
    out = np.stack([res.results[b]["out"] for b in range(B)])
    return np.ascontiguousarray(out.reshape(B, C, HH, WW).astype(np.float32))


# revision 42
# speedup vs baseline: 4118.6721x; 4118.6721x over previous
"""Trainium2 Bass kernel for nn_CDA_attention (density-modulated attention).

Contract: kernel(**full_inputs) -> full output [8, 256, 64, 64] float32.
Data-parallel over batch: core b computes batch b.

Per-core computation (batch b, C=256, N=4096):
  - density chain (Laplacian -> conv(1->8) -> relu -> conv(8->1) -> sigmoid)
    in [64,64] image layout on DVE/ACT, producing per-key scale
    skv[nk] = C^-0.5 / (1 + 2*(1-density)).
  - q,k = Wq/Wk @ x  (float32r matmuls, [c,n] layout)
  - vproj = (out_w @ Wv) @ x, transposed layout [nk, c] with a ones column
    appended so attn@vproj also yields softmax row-sums.
  - scores computed transposed: sT[nk, nq] = k^T q; exp fused with the
    per-key scale as a per-partition activation scale; softmax division by
    the row-sum after the attn@vproj matmul.
  - final transpose back to [c, n] on the PE, + fused bias + residual.
"""

import os
import sys

sys.path.insert(0, "/opt/trn_rl_repo")

from contextlib import ExitStack

import numpy as np

import concourse.bass as bass
import concourse.mybir as mybir
import concourse.tile as tile
from concourse import bacc, bass_utils
from concourse.masks import make_identity

B, C, HH, WW = 8, 256, 64, 64
N = HH * WW          # 4096
P = 128
CC = C // P          # 2 channel chunks
NQT = 512            # query tile (columns per QK^T matmul)
NQ_TILES = N // NQT  # 8
NKC = N // P         # 32 key chunks
NSUB = NQT // P      # 4 query sub-tiles per query tile

f32 = mybir.dt.float32
f32r = mybir.dt.float32r
bf16 = mybir.dt.bfloat16
AF = mybir.ActivationFunctionType
ALU = mybir.AluOpType

# tap order for 3x3 convs: center first so the first tap writes the full tile
TAPS = [(1, 1)] + [(ky, kx) for ky in range(3) for kx in range(3) if (ky, kx) != (1, 1)]


def _make_row_shifted(nc, pool, src, name):
    """Return {dy: AP} of row-shifted copies of src ([64, ...] SBUF tile):
    sh[+1][p] = src[p+1] (last row 0), sh[-1][p] = src[p-1] (first row 0).
    Compute engines need 32-aligned partition bases, DMA does not — so the
    row shift is done once by DMA into zeroed tiles and every conv tap then
    reads/writes full partition ranges."""
    shape = list(src.shape)
    p1 = pool.tile(shape, f32, name=f"{name}_p1")
    m1 = pool.tile(shape, f32, name=f"{name}_m1")
    nc.gpsimd.memset(p1[:], 0.0)
    nc.gpsimd.memset(m1[:], 0.0)
    nc.sync.dma_start(p1[0:63], src[1:64])
    nc.sync.dma_start(m1[1:64], src[0:63])
    return {0: src, 1: p1, -1: m1}


def _conv3x3(eng, out_t, var_slices, wcol_fn, ics):
    """out[h,w] = sum over (ic in ics, ky, kx) of w[ic,ky,kx]*in[ic,h+ky-1,w+kx-1].

    var_slices(ic, dy, cs) -> [64, len] AP of the dy-row-shifted variant of
    input channel ic, columns cs.  wcol_fn(ic, t) -> [64,1] weight AP.
    First tap (center) initializes out_t.  eng is the issuing engine
    (nc.vector or nc.gpsimd) so the chain can run split across engines.
    """
    first = True
    for ic in ics:
        for ky, kx in TAPS:
            dy, dx = ky - 1, kx - 1
            c0, c1 = max(0, -dx), WW - max(0, dx)
            src = var_slices(ic, dy, slice(c0 + dx, c1 + dx))
            wcol = wcol_fn(ic, ky * 3 + kx)
            if first:
                assert (dy, dx) == (0, 0)
                eng.tensor_scalar(
                    out=out_t[:, :], in0=src, scalar1=wcol, scalar2=None,
                    op0=ALU.mult)
                first = False
            else:
                dst = out_t[:, c0:c1]
                eng.scalar_tensor_tensor(
                    out=dst, in0=src, scalar=wcol, in1=dst,
                    op0=ALU.mult, op1=ALU.add)


def build_kernel_body(tc, ctx, d):
    nc = tc.nc
    x_d, wqk_d, wvo_d, qkb_d, bfin_d = d["x"], d["wqk"], d["wvo"], d["qkb"], d["bfin"]
    w1b_d, w1bias_d, w2b_d, w2bias_d = d["w1b"], d["w1bias"], d["w2b"], d["w2bias"]
    out_d, scr1, scr2 = d["out"], d["scr1"], d["scr2"]

    const = ctx.enter_context(tc.tile_pool(name="const", bufs=1))
    big = ctx.enter_context(tc.tile_pool(name="big", bufs=1))
    ps_pool = ctx.enter_context(tc.tile_pool(name="ps", bufs=2, space="PSUM"))
    po_pool = ctx.enter_context(tc.tile_pool(name="po", bufs=4, space="PSUM"))
    fin_pool = ctx.enter_context(tc.tile_pool(name="fin", bufs=2))
    osb_pool = ctx.enter_context(tc.tile_pool(name="osb", bufs=2))
    rcp_pool = ctx.enter_context(tc.tile_pool(name="rcp", bufs=4))
    qt_pool = ctx.enter_context(tc.tile_pool(name="qt", bufs=2))
    repl_pool = ctx.enter_context(tc.tile_pool(name="repl", bufs=2))
    grow_pool = ctx.enter_context(tc.tile_pool(name="grow", bufs=2))

    # ---- persistent SBUF tiles ----
    # x is split into quarters (separate tiles) so consumers can start as
    # soon as their quarter's DMA lands instead of waiting for all 4 MB.
    XQ = N // 4
    x_parts = [big.tile([P, CC, XQ], f32r, name=f"xp{t}") for t in range(4)]

    def x_slice(ci, start, size):
        t = start // XQ
        assert (start + size - 1) // XQ == t
        o = start - t * XQ
        return x_parts[t][:, ci, o:o + size]

    k_sb = big.tile([P, CC, N], f32r)
    vproj_sb = big.tile([P, NKC, C + 1], bf16)
    exp_a = big.tile([P, NKC, NQT], bf16)
    exp_b = big.tile([P, NKC, NQT], bf16)
    wqk_sb = const.tile([P, CC, 2 * C], f32r)
    wvo_sb = const.tile([P, CC, C], f32r)
    qkb_sb = const.tile([P, 4], f32)
    bfin_sb = const.tile([P, 2], f32)
    w1b_sb = const.tile([64, 9, 8, 1], f32)   # [tap, oc] weight patterns
    w1bias_sb = const.tile([64, 8, 1], f32)
    w2b_sb = const.tile([64, 9, 8, 1], f32)   # [tap, ic]
    w2bias_sb = const.tile([64, 1], f32)
    ident = const.tile([P, P], f32)
    ones_sb = const.tile([P, 1], f32r)
    gray_img = const.tile([64, 64], f32)
    lap_t = const.tile([64, 64], f32)
    abs_t = const.tile([64, 1, 64], f32)
    h1_t = const.tile([64, 8, 64], f32)
    h1r_t = const.tile([64, 8, 64], f32)
    cacc_t = const.tile([64, 8, 64], f32)
    ctmp_t = const.tile([64, 8, 64], f32)
    dl_t = const.tile([64, 64], f32)
    sig_t = const.tile([64, 64], f32)
    skv_t = const.tile([64, 64], f32)

    # ---- input DMAs (weights first, then x quarter by quarter) ----
    for ci in range(CC):
        nc.sync.dma_start(wqk_sb[:, ci, :], wqk_d[ci * P:(ci + 1) * P, :])
        nc.sync.dma_start(wvo_sb[:, ci, :], wvo_d[ci * P:(ci + 1) * P, :])
    nc.sync.dma_start(ones_sb[:, :], d["ones"][:, :])
    nc.sync.dma_start(qkb_sb[:, :], qkb_d[:, :])
    nc.sync.dma_start(bfin_sb[:, :], bfin_d[:, :])
    nc.sync.dma_start(w1b_sb[:, :, :, 0], w1b_d.rearrange("p (t o) -> p t o", o=8))
    nc.sync.dma_start(w1bias_sb[:, :, 0], w1bias_d[:, :])
    nc.sync.dma_start(w2b_sb[:, :, :, 0], w2b_d.rearrange("p (t o) -> p t o", o=8))
    nc.sync.dma_start(w2bias_sb[:, :], w2bias_d[:, :])
    for t in range(4):
        for ci in range(CC):
            nc.sync.dma_start(
                x_parts[t][:, ci, :], x_d[ci * P:(ci + 1) * P, t * XQ:(t + 1) * XQ])

    make_identity(nc, ident)
    nc.gpsimd.memset(vproj_sb[:, :, C:C + 1], 1.0)    # ones column -> row sums

    # ---- gray = mean_c x  (PE, M=1) ----
    scr1_2d = scr1.rearrange("(a b) -> a b", a=1)
    for nt in range(NQ_TILES):
        pg = ps_pool.tile([1, NQT], f32, tag="ps")
        for ci in range(CC):
            nc.tensor.matmul(
                pg[:, :],
                ones_sb[:, :],
                x_slice(ci, nt * NQT, NQT),
                start=(ci == 0), stop=(ci == CC - 1))
        grow = grow_pool.tile([1, NQT], f32)
        nc.vector.tensor_copy(grow[:, :], pg[:, :])
        nc.sync.dma_start(scr1_2d[:, nt * NQT:(nt + 1) * NQT], grow[:, :])

    # reshape [1, 4096] -> [64, 64] via DRAM
    nc.sync.dma_start(gray_img[:, :], scr1.rearrange("(h w) -> h w", w=64))

    # ---- k projection (float32r) ----
    # Emitted BEFORE the density chain: the DVE is in-order, so the psum
    # evictions must sit ahead of the ~150 small density-chain ops in its
    # queue or the projection matmuls stall on full PSUM pools.
    for m in (2, 3):                     # c_out chunks k0, k1
        mm = m % 2
        for nt in range(NQ_TILES):
            pq = ps_pool.tile([P, NQT], f32, tag="ps")
            for ci in range(CC):
                nc.tensor.matmul(
                    pq[:, :],
                    wqk_sb[:, ci, m * P:(m + 1) * P],
                    x_slice(ci, nt * NQT, NQT),
                    start=(ci == 0), stop=(ci == CC - 1))
            # evict on the (otherwise idle) scalar engine so the DVE can run
            # the density chain concurrently with the projections
            nc.scalar.activation(
                k_sb[:, mm, nt * NQT:(nt + 1) * NQT], pq[:, :], AF.Identity,
                bias=qkb_sb[:, m:m + 1])

    # ---- vproj = (Wout @ Wv) x, transposed [nk, c] (float32r -> bf16) ----
    for j in range(NKC):
        pv = po_pool.tile([P, C], f32, tag="po")
        for ci in range(CC):
            nc.tensor.matmul(
                pv[:, :],
                x_slice(ci, j * P, P),
                wvo_sb[:, ci, :],
                start=(ci == 0), stop=(ci == CC - 1))
        nc.scalar.activation(vproj_sb[:, j, 0:C], pv[:, :], AF.Copy)

    # ---- density chain (image layout, DVE + ACT; emitted after the
    # projection evictions so it doesn't block them in the DVE queue) ----
    # Laplacian: 4*g - up - down - left - right (zero SAME padding)
    gvar = _make_row_shifted(nc, const, gray_img, "gray")
    nc.vector.tensor_scalar(
        out=lap_t[:, :], in0=gray_img[:, :], scalar1=4.0, scalar2=None, op0=ALU.mult)
    for dy in (1, -1):  # out[h] += -g[h+dy]
        nc.vector.scalar_tensor_tensor(
            out=lap_t[:, :], in0=gvar[dy][:, :], scalar=-1.0, in1=lap_t[:, :],
            op0=ALU.mult, op1=ALU.add)
    for dx in (1, -1):
        c0, c1 = max(0, -dx), WW - max(0, dx)
        dst = lap_t[:, c0:c1]
        nc.vector.scalar_tensor_tensor(
            out=dst, in0=gray_img[:, c0 + dx:c1 + dx], scalar=-1.0, in1=dst,
            op0=ALU.mult, op1=ALU.add)
    nc.scalar.activation(abs_t[:, 0, :], lap_t[:, :], AF.Abs)

    # conv1: 1 -> 8 channels, all channels per tap in one wide op via
    # free-dim-broadcast APs (image broadcast over oc x weight broadcast
    # over w), two DVE ops per tap instead of two per (oc, tap).
    avar = _make_row_shifted(nc, const, abs_t, "abs")

    def conv_taps(out_t, in_var, wpat):
        for i, (ky, kx) in enumerate(TAPS):
            dy, dx = ky - 1, kx - 1
            c0, c1 = max(0, -dx), WW - max(0, dx)
            L = c1 - c0
            src = in_var(dy, slice(c0 + dx, c1 + dx))
            w = wpat[:, ky * 3 + kx, :, :].broadcast_to([64, 8, L])
            if i == 0:
                assert (dy, dx) == (0, 0)
                nc.vector.tensor_mul(out_t[:, :, :], src, w)
            else:
                nc.vector.tensor_mul(ctmp_t[:, :, 0:L], src, w)
                nc.vector.tensor_add(
                    out_t[:, :, c0:c1], out_t[:, :, c0:c1], ctmp_t[:, :, 0:L])

    conv_taps(
        h1_t,
        lambda dy, cs: avar[dy][:, :, cs].broadcast_to(
            [64, 8, cs.stop - cs.start]),
        w1b_sb)
    # relu(h1 + bias) in two wide ops
    nc.vector.tensor_add(
        h1_t[:, :, :], h1_t[:, :, :], w1bias_sb.broadcast_to([64, 8, WW]))
    nc.vector.tensor_scalar(
        out=h1r_t[:, :, :], in0=h1_t[:, :, :], scalar1=0.0, scalar2=None,
        op0=ALU.max)

    # conv2: 8 -> 1 channel: per-tap wide ops into per-ic partial sums,
    # then a 3-step tree reduction over ic; sigmoid with bias
    hvar = _make_row_shifted(nc, const, h1r_t, "h1r")
    conv_taps(cacc_t, lambda dy, cs: hvar[dy][:, :, cs], w2b_sb)
    nc.vector.tensor_add(cacc_t[:, 0:4, :], cacc_t[:, 0:4, :], cacc_t[:, 4:8, :])
    nc.vector.tensor_add(cacc_t[:, 0:2, :], cacc_t[:, 0:2, :], cacc_t[:, 2:4, :])
    nc.vector.tensor_add(dl_t[:, :], cacc_t[:, 0, :], cacc_t[:, 1, :])
    nc.scalar.activation(sig_t[:, :], dl_t[:, :], AF.Sigmoid, bias=w2bias_sb[:, 0:1])
    # skv = C^-0.5 / (3 - 2*sigmoid)
    nc.scalar.activation(dl_t[:, :], sig_t[:, :], AF.Copy, bias=3.0, scale=-2.0)
    nc.vector.reciprocal(sig_t[:, :], dl_t[:, :])
    nc.vector.tensor_scalar(
        out=skv_t[:, :], in0=sig_t[:, :], scalar1=float(C) ** -0.5, scalar2=None,
        op0=ALU.mult)
    # skv (with C^-0.5 folded) -> DRAM, flat [4096] keyed by n = h*64+w
    nc.sync.dma_start(scr2.rearrange("(h w) -> h w", w=64), skv_t[:, :])

    # ---- pre-scale k by the per-key softmax scale (k *= skv[nk]) ----
    # The scale multiplies whole score columns, so folding it into k makes
    # the exp activation scale-free (batchable across PSUM banks).
    scr2_1 = scr2.rearrange("(a b) -> a b", a=1)
    for nt in range(NQ_TILES):
        sl = slice(nt * NQT, (nt + 1) * NQT)
        repl = repl_pool.tile([P, NQT], f32)
        nc.sync.dma_start(repl[:, :], scr2_1[0:1, sl].broadcast_to([P, NQT]))
        for ci in range(CC):
            nc.vector.tensor_mul(
                k_sb[:, ci, sl], k_sb[:, ci, sl].bitcast(f32), repl[:, :])

    # ---- attention ----
    for it in range(NQ_TILES):
        nq0 = it * NQT
        exp_sb = exp_a if it % 2 == 0 else exp_b

        # q tile projection (just-in-time, float32r)
        q_t = qt_pool.tile([P, CC, NQT], f32r)
        for mm in range(CC):
            pq = ps_pool.tile([P, NQT], f32, tag="ps")
            for ci in range(CC):
                nc.tensor.matmul(
                    pq[:, :],
                    wqk_sb[:, ci, mm * P:(mm + 1) * P],
                    x_slice(ci, nq0, NQT),
                    start=(ci == 0), stop=(ci == CC - 1))
            nc.vector.tensor_scalar(
                out=q_t[:, mm, :], in0=pq[:, :],
                scalar1=qkb_sb[:, mm:mm + 1], scalar2=None, op0=ALU.add)

        # Scores (transposed) in key-chunk PAIRS — one exp activation covers
        # two PSUM banks (halves the ACT per-element overhead) — with the
        # attn@V matmuls for the PREVIOUS pair interleaved in PE program
        # order. Per step the PE issues 4 QK^T + 8 attn@V matmuls (~1.8us)
        # while ACT exponentiates one pair (~1.2us): the in-order PE never
        # waits on the exp handoff and HAM stays at full clock.
        pos = [po_pool.tile([P, C + 1], f32, tag="po", name=f"po{s}")
               for s in range(NSUB)]

        def attnv_chunk(j):
            for s in range(NSUB):
                nc.tensor.matmul(
                    pos[s][:, :],
                    exp_sb[:, j, s * P:(s + 1) * P],
                    vproj_sb[:, j, :],
                    start=(j == 0), stop=(j == NKC - 1))

        for jj in range(NKC // 2):
            ps2 = ps_pool.tile([P, 2, NQT], f32, tag="ps")
            for u in range(2):
                j = 2 * jj + u
                for ci in range(CC):
                    nc.tensor.matmul(
                        ps2[:, u, :],
                        k_sb[:, ci, j * P:(j + 1) * P],
                        q_t[:, ci, :],
                        start=(ci == 0), stop=(ci == CC - 1))
            nc.scalar.activation(
                exp_sb[:, 2 * jj:2 * jj + 2, :], ps2[:, :, :], AF.Exp)
            if jj >= 1:
                attnv_chunk(2 * jj - 2)
                attnv_chunk(2 * jj - 1)
        attnv_chunk(NKC - 2)
        attnv_chunk(NKC - 1)

        fin = fin_pool.tile([P, CC, NQT], f32)
        for s in range(NSUB):
            rcp = rcp_pool.tile([P, 1], f32)
            nc.vector.reciprocal(rcp[:, :], pos[s][:, C:C + 1])
            osb = osb_pool.tile([P, C], f32)
            nc.vector.tensor_scalar(   # softmax normalization (divide by rowsum)
                out=osb[:, :], in0=pos[s][:, 0:C], scalar1=rcp[:, :], scalar2=None,
                op0=ALU.mult)
            for ci in range(CC):
                pt = po_pool.tile([P, P], f32, tag="po", name="pt")
                nc.tensor.transpose(pt[:, :], osb[:, ci * P:(ci + 1) * P], ident[:, :])
                nc.vector.scalar_tensor_tensor(
                    out=fin[:, ci, s * P:(s + 1) * P],
                    in0=pt[:, :],
                    scalar=bfin_sb[:, ci:ci + 1],
                    in1=x_slice(ci, nq0 + s * P, P).bitcast(f32),
                    op0=ALU.add, op1=ALU.add)
        for ci in range(CC):
            nc.sync.dma_start(out_d[ci * P:(ci + 1) * P, nq0:nq0 + NQT], fin[:, ci, :])


def build_nc():
    nc = bacc.Bacc("TRN2", target_bir_lowering=False, debug=False)
    d = {}
    def inp(name, shape, dt=f32):
        d[name] = nc.dram_tensor(name, shape, dt, kind="ExternalInput").ap()
    inp("x", (C, N), f32r)      # f32r so a plain DMA is a legal f32r producer
    inp("wqk", (C, 2 * C), f32r)
    inp("wvo", (C, C), f32r)
    inp("ones", (P, 1), f32r)   # 1/C column (folds the channel mean)
    inp("qkb", (P, 4))
    inp("bfin", (P, 2))
    inp("w1b", (64, 72))
    inp("w1bias", (64, 8))
    inp("w2b", (64, 72))
    inp("w2bias", (64, 1))
    d["out"] = nc.dram_tensor("out", (C, N), f32, kind="ExternalOutput").ap()
    d["scr1"] = nc.dram_tensor("scr1", (N,), f32, kind="Internal").ap()
    d["scr2"] = nc.dram_tensor("scr2", (N,), f32, kind="Internal").ap()

    with tile.TileContext(nc) as tc, ExitStack() as ctx:
        build_kernel_body(tc, ctx, d)
    nc.compile()
    return nc


def host_inputs(x, qkv_w, qkv_b, out_w, out_b, d1_w, d1_b, d2_w, d2_b):
    f = np.float32
    x = np.asarray(x, f)
    wq = np.asarray(qkv_w, f)[:, :, 0, 0]          # [768, 256]
    qkv_b = np.asarray(qkv_b, f)
    wout = np.asarray(out_w, f)[:, :, 0, 0]        # [256, 256]
    out_b = np.asarray(out_b, f)
    shared = {
        "wqk": np.ascontiguousarray(wq[0:2 * C].T),
        "wvo": np.ascontiguousarray((wout @ wq[2 * C:3 * C]).T),
        "qkb": np.ascontiguousarray(qkv_b[0:2 * C].reshape(4, P).T),
        "bfin": np.ascontiguousarray(
            (wout @ qkv_b[2 * C:3 * C] + out_b).reshape(2, P).T),
        # tap-major [t*8 + ch] weight patterns for the wide conv ops
        "w1b": np.tile(
            np.ascontiguousarray(np.asarray(d1_w, f).reshape(8, 9).T).reshape(1, 72),
            (64, 1)),
        "w1bias": np.tile(np.asarray(d1_b, f).reshape(1, 8), (64, 1)),
        "w2b": np.tile(
            np.ascontiguousarray(np.asarray(d2_w, f).reshape(8, 9).T).reshape(1, 72),
            (64, 1)),
        "w2bias": np.tile(np.asarray(d2_b, f).reshape(1, 1), (64, 1)),
        "ones": np.full((P, 1), 1.0 / C, f),
    }
    shared = {k: np.ascontiguousarray(v, dtype=f) for k, v in shared.items()}
    xs = x.reshape(B, C, N)
    return [dict(x=np.ascontiguousarray(xs[b]), **shared) for b in range(B)]


_NC_CACHE = {}


def _get_nc():
    if "nc" not in _NC_CACHE:
        _NC_CACHE["nc"] = build_nc()
    return _NC_CACHE["nc"]


def kernel(x, qkv_w, qkv_b, out_w, out_b, d1_w, d1_b, d2_w, d2_b):
    in_maps = host_inputs(x, qkv_w, qkv_b, out_w, out_b, d1_w, d1_b, d2_w, d2_b)
    nc = _get_nc()
    trace = bool(int(os.environ.get("KERNEL_TRACE", "0")))
    res = bass_utils.run_bass_kernel_spmd(
        nc, in_maps, core_ids=list(range(B)), trace=trace)
    _NC_CACHE["last_results"] = res
    out = np.stack([res.results[b]["out"] for b in range(B)])
    return np.ascontiguousarray(out.reshape(B, C, HH, WW).astype(np.float32))
